# revision 1
# baseline (speedup 1.0000x reference)
"""GCN encoder (2-layer PyG-style GCNConv) as a Bass/Tile kernel on 8 trn2 NeuronCores.

Strategy (graph/data parallel, per sharding hint):
  - Nodes are partitioned across the 8 cores (12544 padded nodes each); each core
    aggregates all edges whose *destination* lands in its shard.
  - Aggregation is computed per 128-node destination block as a sequence of PE
    matmuls: for each tile of 128 edges, gather the 128 source-feature rows with
    `dma_gather` (bf16), build the 128x128 0/1 dst-indicator on the fly
    (iota `is_equal` dst_sel), and accumulate indicator-weighted messages into
    PSUM:  aggT[f,d] += msg[e,f]^T @ ind[e,d].
  - GCN normalization deg^-1/2 A deg^-1/2 is folded as:  table rows are
    pre-scaled by dinv[src] (host for x, on-device for h1) and dinv[dst] is a
    per-partition scalar applied to the block output after the dense transform.
  - Layer 2 needs h1 for all nodes: the scaled hidden table is AllGathered in 4
    node-quarter chunks (overlappable with compute); layer-2 gathers then read
    the gathered tables per chunk.
"""

import sys

sys.path.insert(0, "/opt/trn_rl_repo")

import numpy as np

import concourse.bass as bass
import concourse.bacc as bacc
import concourse.mybir as mybir
from concourse import tile, library_config

BF16 = mybir.dt.bfloat16
F32 = mybir.dt.float32
I16 = mybir.dt.int16
BF16_NP = mybir.dt.np(BF16)

DIN, DH, DOUT = 128, 128, 64


def make_cfg(n_nodes, n_edges, n_cores=8, bpc=98, bpg=7, q_blocks=(25, 25, 24, 24),
             gcap=896, n_queues=2):
    cfg = {}
    cfg["N"] = n_nodes
    cfg["E"] = n_edges
    cfg["GCAP"] = gcap          # max indices per dma_gather instruction
    cfg["NQ"] = n_queues        # SWDGE queues to spread gathers over
    cfg["NCORES"] = n_cores
    cfg["BPC"] = bpc                      # dst blocks (of 128 nodes) per core
    cfg["BPG"] = bpg                      # blocks per gather group
    assert bpc % bpg == 0
    cfg["NG"] = bpc // bpg                # gather groups per core
    cfg["SHARD"] = bpc * 128              # padded nodes per core
    cfg["NP"] = n_cores * cfg["SHARD"]    # padded total nodes
    assert cfg["NP"] >= n_nodes
    assert cfg["NP"] % 4 == 0
    cfg["CH1"] = cfg["NP"] // 4           # layer-1 gather chunk (by raw node id)
    assert cfg["CH1"] <= 32767
    assert sum(q_blocks) == bpc and len(q_blocks) == 4
    cfg["QB"] = list(q_blocks)            # blocks per quarter (collective chunks)
    cfg["QSTART"] = np.concatenate([[0], np.cumsum(q_blocks)])  # block ids
    cfg["QN"] = [q * 128 for q in q_blocks]   # nodes per quarter per rank
    for q in q_blocks:
        assert q * 128 * n_cores <= 32767
    return cfg


def _block_quarter(cfg, blk):
    """quarter id for a block index (vectorized)."""
    return np.searchsorted(cfg["QSTART"][1:], blk, side="right")


def make_layout(cfg, L):
    """Static slot/position layout from the padded per-(block, chunk) length
    table L [BPC, 4] (multiples of 128, identical across cores).

    Global ordering: group-major, then chunk, then block within group.
    Returns dict with position bases and group extents."""
    BPC, BPG, NG = cfg["BPC"], cfg["BPG"], cfg["NG"]
    gpos = np.zeros((BPC, 4), np.int64)      # global position base of run (b, c)
    run_len = np.zeros((NG, 4), np.int64)    # positions per (g, c) gather
    grp_base = np.zeros(NG + 1, np.int64)    # global position base of group g
    p = 0
    for g in range(NG):
        grp_base[g] = p
        for c in range(4):
            for b in range(g * BPG, (g + 1) * BPG):
                gpos[b, c] = p
                p += L[b, c]
            run_len[g, c] = p - (gpos[g * BPG, c])
    grp_base[NG] = p
    return {
        "gpos": gpos,
        "run_len": run_len,
        "grp_base": grp_base,
        "total_pos": p,
        "total_slots": p // 128,
    }


def preprocess(cfg, x, edge_index, W1, b1, W2, b2):
    """Host-side sharding: bucket/sort edges, build per-core gather index and
    dst-selector streams, degree normalization, bf16 tables."""
    N, NP, NC = cfg["N"], cfg["NP"], cfg["NCORES"]
    SHARD, BPC = cfg["SHARD"], cfg["BPC"]
    CH1 = cfg["CH1"]

    x = np.asarray(x, np.float32)
    edge_index = np.asarray(edge_index)
    W1 = np.asarray(W1, np.float32)
    b1 = np.asarray(b1, np.float32)
    W2 = np.asarray(W2, np.float32)
    b2 = np.asarray(b2, np.float32)

    loops = np.arange(N, dtype=np.int64)
    src = np.concatenate([edge_index[0].astype(np.int64), loops])
    dst = np.concatenate([edge_index[1].astype(np.int64), loops])

    deg = np.bincount(dst, minlength=NP).astype(np.float32)
    dinv = np.zeros(NP, np.float32)
    nz = deg > 0
    dinv[nz] = 1.0 / np.sqrt(deg[nz])

    # layer-1 gather table: dinv[src]-prescaled features, bf16 (raw node order)
    xs = np.zeros((NP, DIN), np.float32)
    xs[:N] = x * dinv[:N, None]
    xt = xs.astype(BF16_NP)

    # degree-balanced node -> (core, block, slot) packing: serpentine deal of
    # nodes sorted by in-degree so every 128-node block has ~equal edge count
    NB = NP // 128
    order = np.argsort(-deg[:N], kind="stable")
    ids = np.concatenate([order, np.full(NP - N, -1, np.int64)])
    rounds = ids.reshape(128, NB).copy()
    rounds[1::2] = rounds[1::2, ::-1]
    posmat = (np.arange(NB)[None, :] * 128 + np.arange(128)[:, None])
    node_pos = np.zeros(N, np.int64)
    m = rounds >= 0
    node_pos[rounds[m]] = posmat[m]

    p_dst = node_pos[dst]
    core = (p_dst // SHARD).astype(np.int32)
    blk = ((p_dst % SHARD) // 128).astype(np.int32)   # block within core
    dloc = (p_dst % 128).astype(np.int32)
    grp = blk // cfg["BPG"]

    # layer-1 chunk: raw node-id range; layer-2 chunk: quarter-major table of
    # packed positions
    c1 = (src // CH1).astype(np.int32)
    p_src = node_pos[src]
    s_rank = p_src // SHARD
    s_loc = p_src % SHARD
    s_blk = (s_loc // 128).astype(np.int32)
    c2 = _block_quarter(cfg, s_blk).astype(np.int32)
    qn = np.asarray(cfg["QN"], np.int64)
    qstart_nodes = cfg["QSTART"][:4] * 128
    pos2 = s_rank * qn[c2] + (s_loc - qstart_nodes[c2])
    idxval1 = (src - c1.astype(np.int64) * CH1).astype(np.int16)
    idxval2 = pos2.astype(np.int16)

    in_maps = [dict() for _ in range(NC)]
    Ls = []
    layouts = []
    for layer, (cl, ival) in enumerate([(c1, idxval1), (c2, idxval2)]):
        # per-core per-(block, chunk) counts -> shared padded length table
        key = (core.astype(np.int64) * BPC + blk) * 4 + cl
        cnt = np.bincount(key, minlength=NC * BPC * 4).reshape(NC, BPC, 4)
        mx = cnt.max(axis=0)
        L = ((mx + 127) // 128) * 128
        # every block needs at least one tile so its PSUM gets written
        empty = L.sum(axis=1) == 0
        L[empty, 0] = 128
        Ls.append(L)
        lay = make_layout(cfg, L)
        layouts.append(lay)

        # stable sort: (core, group, chunk, block, src) ; groups are
        # block-contiguous so (core, chunk-within-group ordering) needs group
        order = np.lexsort((src, blk, cl, grp, core))
        ekey = key[order]
        # position within (core, block, chunk) run (runs are contiguous after sort)
        change = np.r_[True, ekey[1:] != ekey[:-1]]
        starts = np.flatnonzero(change)
        runid = np.cumsum(change) - 1
        within = np.arange(len(ekey)) - starts[runid]
        gp = lay["gpos"]  # [BPC, 4]
        b_o = blk[order]
        c_o = cl[order]
        pos = gp[b_o, c_o] + within
        core_o = core[order]

        total = lay["total_pos"]
        gp_flat = lay["gpos"].reshape(-1)
        L_flat = L.reshape(-1)
        for r in range(NC):
            m = core_o == r
            iarr = np.zeros(total, np.int16)
            sarr = np.full(total, -1.0, np.float32)
            iarr[pos[m]] = ival[order][m]
            sarr[pos[m]] = dloc[order][m].astype(np.float32)
            # forward-fill pad positions with the run's first real index so
            # pad gathers hit nearby/cached table rows
            cnt_r = cnt[r].reshape(-1)
            has = cnt_r > 0
            firsts = np.zeros(len(L_flat), np.int16)
            firsts[has] = iarr[gp_flat[has]]
            ordr = np.argsort(gp_flat, kind="stable")
            run_of_pos = np.repeat(ordr, L_flat[ordr])
            off_of_pos = np.arange(total) - np.repeat(gp_flat[ordr], L_flat[ordr])
            padmask = off_of_pos >= cnt_r[run_of_pos]
            iarr[padmask] = firsts[run_of_pos[padmask]]
            iw = np.tile(np.ascontiguousarray(iarr.reshape(-1, 16).T), (8, 1))
            sw = np.ascontiguousarray(sarr.reshape(-1, 128).T)
            in_maps[r][f"idx{layer + 1}"] = np.ascontiguousarray(iw)
            in_maps[r][f"sel{layer + 1}"] = sw

    iota_np = np.tile(np.arange(128, dtype=np.float32), (128, 1)).astype(BF16_NP)
    w1s = W1.astype(BF16_NP)
    w2s = np.zeros((DH, DOUT), np.float32).astype(BF16_NP)
    w2s[:, :] = W2.astype(BF16_NP)
    b1b = np.tile(b1, (128, 1)).astype(np.float32)
    b2b = np.tile(b2, (128, 1)).astype(np.float32)

    dinv_by_pos = np.zeros(NP, np.float32)
    dinv_by_pos[node_pos] = dinv[:N]
    for r in range(NC):
        sh = dinv_by_pos[r * SHARD:(r + 1) * SHARD]
        in_maps[r]["dinv"] = np.ascontiguousarray(sh.reshape(BPC, 128).T)
        in_maps[r]["xt"] = xt
        in_maps[r]["w1s"] = w1s
        in_maps[r]["w2s"] = w2s
        in_maps[r]["b1b"] = b1b
        in_maps[r]["b2b"] = b2b
        in_maps[r]["iota"] = iota_np

    return in_maps, Ls, layouts, node_pos


def build_nc(cfg, Ls, layouts, debug=False, sim_single=False):
    NC, BPC, BPG, NG = cfg["NCORES"], cfg["BPC"], cfg["BPG"], cfg["NG"]
    SHARD, CH1 = cfg["SHARD"], cfg["CH1"]
    QB, QN, QSTART = cfg["QB"], cfg["QN"], cfg["QSTART"]

    nc = bacc.Bacc("TRN2", target_bir_lowering=False, debug=debug,
                   num_devices=1 if sim_single else NC,
                   num_swdge_queues=cfg["NQ"])

    t_xt = nc.dram_tensor("xt", [cfg["NP"], DIN], BF16, kind="ExternalInput")
    t_w1 = nc.dram_tensor("w1s", [DIN, DH], BF16, kind="ExternalInput")
    t_w2 = nc.dram_tensor("w2s", [DH, DOUT], BF16, kind="ExternalInput")
    t_b1 = nc.dram_tensor("b1b", [128, DH], F32, kind="ExternalInput")
    t_b2 = nc.dram_tensor("b2b", [128, DOUT], F32, kind="ExternalInput")
    t_iota = nc.dram_tensor("iota", [128, 128], BF16, kind="ExternalInput")
    t_dinv = nc.dram_tensor("dinv", [128, BPC], F32, kind="ExternalInput")
    t_idx = []
    t_sel = []
    for layer in (0, 1):
        lay = layouts[layer]
        t_idx.append(nc.dram_tensor(f"idx{layer + 1}", [128, lay["total_pos"] // 16],
                                    I16, kind="ExternalInput"))
        t_sel.append(nc.dram_tensor(f"sel{layer + 1}", [128, lay["total_slots"]],
                                    F32, kind="ExternalInput"))
    t_out = nc.dram_tensor("out", [SHARD, DOUT], F32, kind="ExternalOutput")

    max_grp_pos = max(
        int((lay["grp_base"][g + 1] - lay["grp_base"][g]))
        for lay in layouts for g in range(NG)
    )

    with tile.TileContext(nc) as tc:
        with (
            tc.tile_pool(name="const", bufs=1) as constp,
            tc.tile_pool(name="dram", bufs=1, space="DRAM") as dramp,
            tc.tile_pool(name="idxs", bufs=3) as idxp,
            tc.tile_pool(name="msg", bufs=3) as msgp,
            tc.tile_pool(name="ind", bufs=6) as indp,
            tc.tile_pool(name="aggps", bufs=2, space="PSUM") as aggpsp,
            tc.tile_pool(name="xfps", bufs=2, space="PSUM") as xfpsp,
            tc.tile_pool(name="post", bufs=4) as postp,
        ):
            nc.gpsimd.load_library(library_config.mlp)

            IOTA = constp.tile([128, 128], BF16)
            nc.sync.dma_start(IOTA[:], t_iota[:, :])
            W1 = constp.tile([DIN, DH], BF16)
            nc.sync.dma_start(W1[:], t_w1[:, :])
            W2 = constp.tile([DH, DOUT], BF16)
            nc.sync.dma_start(W2[:], t_w2[:, :])
            B1 = constp.tile([128, DH], F32)
            nc.sync.dma_start(B1[:], t_b1[:, :])
            B2 = constp.tile([128, DOUT], F32)
            nc.sync.dma_start(B2[:], t_b2[:, :])
            DINV = constp.tile([128, BPC], F32)
            nc.sync.dma_start(DINV[:], t_dinv[:, :])
            SEL = []
            for layer in (0, 1):
                s = constp.tile([128, layouts[layer]["total_slots"]], F32,
                                name=f"selbuf{layer}")
                nc.sync.dma_start(s[:], t_sel[layer][:, :])
                SEL.append(s)

            h1_mine = [dramp.tile([QN[q], DH], BF16, name=f"h1mine{q}")
                       for q in range(4)]
            h1_tab = [dramp.tile([QN[q] * NC, DH], BF16, addr_space="Shared",
                                 name=f"h1tab{q}") for q in range(4)]

            def do_layer(layer):
                lay = layouts[layer]
                L = Ls[layer]
                gpos = lay["gpos"]
                for g in range(NG):
                    p0 = int(lay["grp_base"][g])
                    p1 = int(lay["grp_base"][g + 1])
                    if p1 == p0:
                        continue
                    idxt = idxp.tile([128, max_grp_pos // 16], I16, tag="idxt")
                    nc.sync.dma_start(idxt[:, : (p1 - p0) // 16],
                                      t_idx[layer][:, p0 // 16: p1 // 16])
                    msg = msgp.tile([128, max_grp_pos // 128, DH], BF16, tag="msg")
                    gq = 0
                    for c in range(4):
                        nidx = int(lay["run_len"][g, c])
                        if nidx == 0:
                            continue
                        rp0 = int(gpos[g * BPG, c])  # global pos of run start
                        if layer == 0:
                            src_ap = t_xt[c * CH1:(c + 1) * CH1, :]
                        else:
                            src_ap = h1_tab[c][:, :]
                        # split into <= GCAP-index gather instructions
                        for s0 in range(0, nidx, cfg["GCAP"]):
                            n = min(cfg["GCAP"], nidx - s0)
                            a0 = rp0 - p0 + s0       # pos offset in group buf
                            nc.gpsimd.dma_gather(
                                out_ap=msg[:, a0 // 128: (a0 + n) // 128, :],
                                in_ap=src_ap,
                                idxs_ap=idxt[:, a0 // 16: (a0 + n) // 16],
                                num_idxs=n,
                                num_idxs_reg=n,
                                elem_size=DH,
                                queue_num=gq % cfg["NQ"],
                            )
                            gq += 1
                    for b in range(g * BPG, (g + 1) * BPG):
                        slots = []
                        for c in range(4):
                            s0 = int(gpos[b, c])
                            for s in range(s0 // 128, (s0 + L[b, c]) // 128):
                                slots.append(s)
                        assert slots
                        psA = aggpsp.tile([DH, 128], F32, tag="aggps")
                        for k, s in enumerate(slots):
                            ind = indp.tile([128, 128], BF16, tag="ind")
                            nc.any.tensor_scalar(
                                ind[:], IOTA[:], SEL[layer][:, s: s + 1], None,
                                mybir.AluOpType.is_equal,
                            )
                            nc.tensor.matmul(
                                psA[:], lhsT=msg[:, s - p0 // 128, :], rhs=ind[:],
                                start=(k == 0), stop=(k == len(slots) - 1),
                            )
                        aggs = postp.tile([DH, 128], BF16, tag="aggs")
                        nc.vector.tensor_copy(aggs[:], psA[:])
                        dcol = DINV[:, b: b + 1]
                        if layer == 0:
                            psH = xfpsp.tile([128, DH], F32, tag="xfps")
                            nc.tensor.matmul(psH[:], lhsT=aggs[:], rhs=W1[:],
                                             start=True, stop=True)
                            tA = postp.tile([128, DH], F32, tag="tA")
                            nc.any.tensor_scalar(tA[:], psH[:], dcol, None,
                                                 mybir.AluOpType.mult)
                            tB = postp.tile([128, DH], F32, tag="tB")
                            nc.any.tensor_tensor(tB[:], tA[:], B1[:],
                                                 mybir.AluOpType.add)
                            h1s = postp.tile([128, DH], BF16, tag="h1s")
                            nc.scalar.activation(
                                h1s[:], tB[:],
                                mybir.ActivationFunctionType.Relu, scale=dcol,
                            )
                            q = int(_block_quarter(cfg, b))
                            r0 = (b - int(QSTART[q])) * 128
                            nc.sync.dma_start(h1_mine[q][r0:r0 + 128, :], h1s[:])
                            if b == int(QSTART[q + 1]) - 1:
                                if sim_single:
                                    # stand-in for the AllGather so the sim
                                    # keeps the layer-2 dependency structure
                                    nc.sync.dma_start(
                                        h1_tab[q][:QN[q], :], h1_mine[q][:, :])
                                else:
                                    nc.gpsimd.collective_compute(
                                        "AllGather",
                                        mybir.AluOpType.bypass,
                                        replica_groups=[list(range(NC))],
                                        ins=[h1_mine[q].opt()],
                                        outs=[h1_tab[q].opt()],
                                    )
                        else:
                            psO = xfpsp.tile([128, DOUT], F32, tag="xfps")
                            nc.tensor.matmul(psO[:], lhsT=aggs[:], rhs=W2[:],
                                             start=True, stop=True)
                            tA = postp.tile([128, DOUT], F32, tag="tA")
                            nc.any.tensor_scalar(tA[:], psO[:], dcol, None,
                                                 mybir.AluOpType.mult)
                            ot = postp.tile([128, DOUT], F32, tag="ot")
                            nc.any.tensor_tensor(ot[:], tA[:], B2[:],
                                                 mybir.AluOpType.add)
                            nc.sync.dma_start(t_out[b * 128:(b + 1) * 128, :], ot[:])

            do_layer(0)
            do_layer(1)

    nc.compile()
    return nc


def kernel(x, edge_index, W1, b1, W2, b2):
    cfg = make_cfg(100000, 1600000)
    in_maps, Ls, layouts, node_pos = preprocess(cfg, x, edge_index, W1, b1, W2, b2)
    nc = build_nc(cfg, Ls, layouts, debug=False)
    from concourse import bass_utils
    res = bass_utils.run_bass_kernel_spmd(
        nc, in_maps, core_ids=list(range(cfg["NCORES"]))
    )
    out = np.concatenate([res.results[r]["out"] for r in range(cfg["NCORES"])], axis=0)
    return np.ascontiguousarray(out[node_pos])



# revision 3
# speedup vs baseline: 1.2046x; 1.2046x over previous
"""GCN encoder (2-layer PyG-style GCNConv) as a Bass/Tile kernel on 8 trn2 NeuronCores.

v2 strategy (graph/data parallel):
  - Nodes partitioned across 8 cores (98 blocks of 128 per core); each core
    aggregates edges whose destination lands in its shard, via per-128-edge
    indicator matmuls: psA[f,d] += msg[e,f]^T @ ind[e,d].
  - BOTH GCN norm factors dinv[src]*dinv[dst] are folded into the indicator
    build: ind = (IOTA == sel) * scale with a host-precomputed per-position
    scale stream, so tables (x, h1) stay unscaled and no per-block scaling
    ops are needed. Bias is folded as a rank-1 matmul into the transform PSUM.
  - L1 self-loops are not gathered; each block adds x[d]*dinv^2[d] @ W1 via
    one matmul against an SBUF-resident transposed table.
  - h1 is AllGathered in 4 quarter chunks (28/28/28/14 blocks); layer 2 runs
    CHUNK-major with a persistent SBUF f32 accumulator per block, so L2 chunk
    c only waits on AllGather c and overlaps L1's tail.
"""

import sys

sys.path.insert(0, "/opt/trn_rl_repo")

import numpy as np

import concourse.bass as bass
import concourse.bacc as bacc
import concourse.mybir as mybir
from concourse import tile, library_config

BF16 = mybir.dt.bfloat16
F32 = mybir.dt.float32
I16 = mybir.dt.int16
BF16_NP = mybir.dt.np(BF16)

DIN, DH, DOUT = 128, 128, 64


def make_cfg(n_nodes, n_edges, n_cores=8, bpc=98, bpg=7, q_blocks=(28, 28, 28, 14),
             gcap=1024, n_queues=4):
    cfg = {}
    cfg["N"] = n_nodes
    cfg["E"] = n_edges
    cfg["GCAP"] = gcap
    cfg["NQ"] = n_queues
    cfg["NCORES"] = n_cores
    cfg["BPC"] = bpc
    cfg["BPG"] = bpg
    assert bpc % bpg == 0
    cfg["NG"] = bpc // bpg
    cfg["SHARD"] = bpc * 128
    cfg["NP"] = n_cores * cfg["SHARD"]
    assert cfg["NP"] >= n_nodes
    assert cfg["NP"] % 4 == 0
    cfg["CH1"] = cfg["NP"] // 4
    assert cfg["CH1"] <= 32767
    assert sum(q_blocks) == bpc and len(q_blocks) == 4
    for q in q_blocks:
        assert q % bpg == 0
        assert q * 128 * n_cores <= 32767
    cfg["QB"] = list(q_blocks)
    cfg["QSTART"] = np.concatenate([[0], np.cumsum(q_blocks)])  # class ids
    cfg["QN"] = [q * 128 for q in q_blocks]
    return cfg


def _block_quarter(cfg, blk):
    return np.searchsorted(cfg["QSTART"][1:], blk, side="right")


def make_layout(cfg, L, chunk_major):
    """Static slot/position layout from padded per-(class, chunk) length
    table L [BPC, 4] (multiples of 128, shared across cores).

    Ordering: layer1 (chunk_major=False): g, then c, then b within g.
              layer2 (chunk_major=True):  c, then g, then b within g.
    Returns position bases per (b, c), per-(g, c) run info, totals."""
    BPC, BPG, NG = cfg["BPC"], cfg["BPG"], cfg["NG"]
    gpos = np.zeros((BPC, 4), np.int64)     # position base of run (b, c)
    run_len = np.zeros((NG, 4), np.int64)   # positions of gather run (g, c)
    run_base = np.zeros((NG, 4), np.int64)  # position base of run (g, c)
    p = 0
    if chunk_major:
        outer = [(c, g) for c in range(4) for g in range(NG)]
    else:
        outer = [(c, g) for g in range(NG) for c in range(4)]
    for c, g in outer:
        run_base[g, c] = p
        for b in range(g * BPG, (g + 1) * BPG):
            gpos[b, c] = p
            p += L[b, c]
        run_len[g, c] = p - run_base[g, c]
    return {
        "gpos": gpos,
        "run_len": run_len,
        "run_base": run_base,
        "total_pos": p,
        "total_slots": p // 128,
        "chunk_major": chunk_major,
    }


def _greedy_classes(vecs, n_classes, cap=8):
    """Assign len(vecs) blocks to n_classes buckets of `cap`, minimizing
    sum over dims of roundup128(max over bucket members).
    vecs: [n_blocks, D] int. Returns class id per block."""
    n = len(vecs)
    assert n == n_classes * cap
    order = np.argsort(-vecs.sum(axis=1), kind="stable")
    bmax = np.zeros((n_classes, vecs.shape[1]), np.int64)
    bcnt = np.zeros(n_classes, np.int64)
    assign = np.zeros(n, np.int64)

    def pad(x):
        return ((x + 127) // 128) * 128

    for i in order:
        v = vecs[i]
        newmax = np.maximum(bmax, v[None, :])
        dcost = (pad(newmax) - pad(bmax)).sum(axis=1).astype(np.float64)
        dcost[bcnt >= cap] = np.inf
        # tie-break: prefer emptier buckets
        j = int(np.argmin(dcost + bcnt * 1e-6))
        assign[i] = j
        bmax[j] = newmax[j]
        bcnt[j] += 1
    return assign


def _refine_nodes(nvec, gblk, nq, NB, T, rng, passes=30):
    """In-place within-quarter node swap refinement: move per-(block, dim)
    loads under the targets T (just below 128-multiples) to kill slot-padding.
    nvec: [N, D] per-node load vectors; gblk: node->block; nq: node quarter."""
    N, D = nvec.shape
    B = np.zeros((NB, D), np.int64)
    np.add.at(B, gblk, nvec)
    Tf = np.asarray(T, np.int64)

    def phi(v):
        w = v - Tf
        np.clip(w, 0, None, out=w)
        return (w * w).sum(axis=1)

    for _ in range(passes):
        idx = rng.permutation(N)
        idx = idx[np.argsort(nq[idx], kind="stable")]
        half = len(idx) // 2
        i, j = idx[0:2 * half:2], idx[1:2 * half:2]
        keep = (nq[i] == nq[j]) & (gblk[i] != gblk[j])
        i, j = i[keep], j[keep]
        a, b = gblk[i], gblk[j]
        d = nvec[j] - nvec[i]
        old = phi(B[a]) + phi(B[b])
        new = phi(B[a] + d) + phi(B[b] - d)
        gain = new - old
        acc = np.flatnonzero(gain < 0)
        if len(acc) == 0:
            continue
        order = acc[np.argsort(gain[acc], kind="stable")]
        seen = np.zeros(NB, bool)
        for k in order:
            ak, bk = a[k], b[k]
            if seen[ak] or seen[bk]:
                continue
            seen[ak] = seen[bk] = True
            B[ak] += d[k]
            B[bk] -= d[k]
            gblk[i[k]], gblk[j[k]] = bk, ak
    return gblk


def preprocess(cfg, x, edge_index, W1, b1, W2, b2):
    """Host-side sharding: pack nodes into (core, class, slot), build gather
    index / selector / scale streams, tables."""
    N, NP, NC = cfg["N"], cfg["NP"], cfg["NCORES"]
    SHARD, BPC, BPG, NG = cfg["SHARD"], cfg["BPC"], cfg["BPG"], cfg["NG"]
    CH1 = cfg["CH1"]
    QB, QSTART = cfg["QB"], np.asarray(cfg["QSTART"])
    qn = np.asarray(cfg["QN"], np.int64)

    x = np.asarray(x, np.float32)
    edge_index = np.asarray(edge_index)
    W1 = np.asarray(W1, np.float32)
    b1 = np.asarray(b1, np.float32)
    W2 = np.asarray(W2, np.float32)
    b2 = np.asarray(b2, np.float32)

    esrc = edge_index[0].astype(np.int64)
    edst = edge_index[1].astype(np.int64)

    deg = (np.bincount(edst, minlength=NP) + 1.0).astype(np.float32)  # + self
    deg[N:] = 0.0
    dinv = np.zeros(NP, np.float32)
    dinv[:N] = 1.0 / np.sqrt(deg[:N])

    # ---- node -> global block (serpentine by in-degree) ----
    NB = NP // 128
    order = np.argsort(-deg[:N], kind="stable")
    ids = np.concatenate([order, np.full(NP - N, -1, np.int64)])
    rounds = ids.reshape(128, NB).copy()
    rounds[1::2] = rounds[1::2, ::-1]
    # gblk[node], gdloc[node]
    gblk = np.zeros(N, np.int64)
    gdloc = np.zeros(N, np.int64)
    m = rounds >= 0
    gblk[rounds[m]] = np.broadcast_to(np.arange(NB)[None, :], rounds.shape)[m]
    gdloc[rounds[m]] = np.broadcast_to(np.arange(128)[:, None], rounds.shape)[m]

    # ---- blocks -> quarters (balance total degree; capacities NC*QB) ----
    blk_deg = np.bincount(gblk[edst], minlength=NB) + 128
    qcap = np.array([q * NC for q in QB])
    qb_order = np.argsort(-blk_deg, kind="stable")
    quarter_of_gblk = np.zeros(NB, np.int64)
    qload = np.zeros(4, np.float64)
    qcnt = np.zeros(4, np.int64)
    for j in qb_order:
        cand = np.where(qcnt < qcap)[0]
        k = cand[np.argmin(qload[cand] / qcap[cand])]
        quarter_of_gblk[j] = k
        qload[k] += blk_deg[j]
        qcnt[k] += 1

    # ---- node swap refinement: balance per-(block, chunk) loads to just
    # under the 128-slot boundaries (within quarters; L1 chunk is intrinsic
    # to the src raw id and L2 chunk labels are quarter-invariant) ----
    c1e = (esrc // CH1).astype(np.int64)
    c2e = quarter_of_gblk[gblk[esrc]]
    nvec = np.zeros((N, 8), np.int64)
    np.add.at(nvec, (edst, c1e), 1)
    np.add.at(nvec, (edst, 4 + c2e), 1)
    rng = np.random.default_rng(12345)
    T = [512, 512, 512, 512, 640, 640, 640, 384]
    gblk = _refine_nodes(nvec, gblk, quarter_of_gblk[gblk], NB, T, rng,
                         passes=60)

    # recompute dloc within refined blocks
    order_n = np.argsort(gblk, kind="stable")
    bstart = np.concatenate([[0], np.cumsum(np.bincount(gblk, minlength=NB))])
    gdloc = np.empty(N, np.int64)
    gdloc[order_n] = np.arange(N) - bstart[gblk[order_n]]
    assert gdloc.max() < 128

    # ---- per-block chunk vectors ----
    c2e = quarter_of_gblk[gblk[esrc]]
    v1 = np.zeros((NB, 4), np.int64)
    np.add.at(v1, (gblk[edst], c1e), 1)
    v2 = np.zeros((NB, 4), np.int64)
    np.add.at(v2, (gblk[edst], c2e), 1)

    # ---- greedy class assignment within each quarter ----
    class_of_gblk = np.zeros(NB, np.int64)
    core_of_gblk = np.zeros(NB, np.int64)
    for q in range(4):
        blks = np.where(quarter_of_gblk == q)[0]
        vec = np.concatenate([v1[blks], v2[blks]], axis=1)
        a = _greedy_classes(vec, QB[q], NC)
        cls = int(QSTART[q]) + a
        class_of_gblk[blks] = cls
        # core = index within class
        for ccls in range(int(QSTART[q]), int(QSTART[q + 1])):
            members = blks[cls == ccls]
            core_of_gblk[members] = np.arange(len(members))

    # ---- node positions ----
    node_core = core_of_gblk[gblk]
    node_cls = class_of_gblk[gblk]
    node_pos = node_core * SHARD + node_cls * 128 + gdloc

    # quarter-local table position for L2 (rank-major per quarter)
    node_q = quarter_of_gblk[gblk]
    qstart_nodes = QSTART[:4] * 128  # class*128 offsets
    node_pos2 = node_core * qn[node_q] + (node_cls - QSTART[node_q]) * 128 + gdloc

    # ---- streams per layer ----
    in_maps = [dict() for _ in range(NC)]
    Ls = []
    layouts = []

    # (stream arrays): both layers exclude self-loops (direct local path)
    streams = []
    streams.append(dict(
        src=esrc, dst=edst,
        cl=c1e,
        ival=(esrc - c1e * CH1).astype(np.int16),
        chunk_major=False,
    ))
    ival2 = node_pos2[esrc]
    assert ival2.max() <= 32767
    streams.append(dict(
        src=esrc, dst=edst, cl=c2e, ival=ival2.astype(np.int16),
        chunk_major=True,
    ))

    for layer, st in enumerate(streams):
        src, dst, cl, ival = st["src"], st["dst"], st["cl"], st["ival"]
        core = node_core[dst]
        blk = node_cls[dst]
        dloc = gdloc[dst]
        grp = blk // BPG

        key = (core * BPC + blk) * 4 + cl
        cnt = np.bincount(key, minlength=NC * BPC * 4).reshape(NC, BPC, 4)
        mx = cnt.max(axis=0)
        L = ((mx + 127) // 128) * 128
        Ls.append(L)
        lay = make_layout(cfg, L, st["chunk_major"])
        layouts.append(lay)

        if st["chunk_major"]:
            order_e = np.lexsort((src, blk, grp, cl, core))
        else:
            order_e = np.lexsort((src, blk, cl, grp, core))
        ekey = key[order_e]
        change = np.r_[True, ekey[1:] != ekey[:-1]]
        starts = np.flatnonzero(change)
        runid = np.cumsum(change) - 1
        within = np.arange(len(ekey)) - starts[runid]
        gp = lay["gpos"]
        pos = gp[blk[order_e], cl[order_e]] + within
        core_o = core[order_e]

        total = lay["total_pos"]
        gp_flat = gp.reshape(-1)
        L_flat = L.reshape(-1)
        scl_all = (dinv[src] * dinv[dst]).astype(np.float32)
        for r in range(NC):
            mrk = core_o == r
            iarr = np.zeros(total, np.int16)
            sarr = np.full(total, -1.0, np.float32)
            carr = np.zeros(total, np.float32)
            iarr[pos[mrk]] = ival[order_e][mrk]
            sarr[pos[mrk]] = dloc[order_e][mrk].astype(np.float32)
            carr[pos[mrk]] = scl_all[order_e][mrk]
            # forward-fill pad positions with the run's first real index
            cnt_r = cnt[r].reshape(-1)
            has = cnt_r > 0
            firsts = np.zeros(len(L_flat), np.int16)
            firsts[has] = iarr[gp_flat[has]]
            ordr = np.argsort(gp_flat, kind="stable")
            run_of_pos = np.repeat(ordr, L_flat[ordr])
            off_of_pos = np.arange(total) - np.repeat(gp_flat[ordr], L_flat[ordr])
            padmask = off_of_pos >= cnt_r[run_of_pos]
            iarr[padmask] = firsts[run_of_pos[padmask]]
            iw = np.tile(np.ascontiguousarray(iarr.reshape(-1, 16).T), (8, 1))
            sw = np.ascontiguousarray(sarr.reshape(-1, 128).T)
            cw = np.ascontiguousarray(carr.reshape(-1, 128).T)
            in_maps[r][f"idx{layer + 1}"] = np.ascontiguousarray(iw)
            in_maps[r][f"sel{layer + 1}"] = sw
            in_maps[r][f"scl{layer + 1}"] = cw

    # ---- tables ----
    xt = np.zeros((NP, DIN), np.float32)
    xt[:N] = x
    xt = xt.astype(BF16_NP)

    # transposed self table, packed order, prescaled by dinv^2
    xself = np.zeros((NP, DIN), np.float32)
    xself[node_pos] = x * (dinv[:N] ** 2)[:, None]
    iota_np = np.tile(np.arange(128, dtype=np.float32), (128, 1)).astype(BF16_NP)
    w1s = W1.astype(BF16_NP)
    w2s = W2.astype(BF16_NP)
    b1r = b1.reshape(1, DH).astype(BF16_NP)
    b2r = b2.reshape(1, DOUT).astype(BF16_NP)
    ones1 = np.ones((1, 128), BF16_NP)

    dinv2_by_pos = np.zeros(NP, np.float32)
    dinv2_by_pos[node_pos] = dinv[:N] ** 2
    selfsel = np.arange(128, dtype=np.float32).reshape(128, 1)
    for r in range(NC):
        sh = xself[r * SHARD:(r + 1) * SHARD]  # [SHARD, DIN]
        in_maps[r]["xselfT"] = np.ascontiguousarray(sh.T.astype(BF16_NP))
        d2 = dinv2_by_pos[r * SHARD:(r + 1) * SHARD]
        in_maps[r]["dinv2"] = np.ascontiguousarray(d2.reshape(BPC, 128).T)
        in_maps[r]["selfsel"] = selfsel
        in_maps[r]["xt"] = xt
        in_maps[r]["w1s"] = w1s
        in_maps[r]["w2s"] = w2s
        in_maps[r]["b1r"] = b1r
        in_maps[r]["b2r"] = b2r
        in_maps[r]["ones1"] = ones1
        in_maps[r]["iota"] = iota_np

    return in_maps, Ls, layouts, node_pos


def build_nc(cfg, Ls, layouts, debug=False, sim_single=False, fake_ag=False):
    NC, BPC, BPG, NG = cfg["NCORES"], cfg["BPC"], cfg["BPG"], cfg["NG"]
    SHARD, CH1 = cfg["SHARD"], cfg["CH1"]
    QB, QN, QSTART = cfg["QB"], cfg["QN"], cfg["QSTART"]

    nc = bacc.Bacc("TRN2", target_bir_lowering=False, debug=debug,
                   num_devices=1 if sim_single else NC,
                   num_swdge_queues=cfg["NQ"])

    t_xt = nc.dram_tensor("xt", [cfg["NP"], DIN], BF16, kind="ExternalInput")
    t_xself = nc.dram_tensor("xselfT", [DIN, SHARD], BF16, kind="ExternalInput")
    t_w1 = nc.dram_tensor("w1s", [DIN, DH], BF16, kind="ExternalInput")
    t_w2 = nc.dram_tensor("w2s", [DH, DOUT], BF16, kind="ExternalInput")
    t_b1 = nc.dram_tensor("b1r", [1, DH], BF16, kind="ExternalInput")
    t_b2 = nc.dram_tensor("b2r", [1, DOUT], BF16, kind="ExternalInput")
    t_ones = nc.dram_tensor("ones1", [1, 128], BF16, kind="ExternalInput")
    t_iota = nc.dram_tensor("iota", [128, 128], BF16, kind="ExternalInput")
    t_dinv2 = nc.dram_tensor("dinv2", [128, BPC], F32, kind="ExternalInput")
    t_selfsel = nc.dram_tensor("selfsel", [128, 1], F32, kind="ExternalInput")
    t_idx = []
    t_sel = []
    t_scl = []
    for layer in (0, 1):
        lay = layouts[layer]
        t_idx.append(nc.dram_tensor(f"idx{layer + 1}", [128, lay["total_pos"] // 16],
                                    I16, kind="ExternalInput"))
        t_sel.append(nc.dram_tensor(f"sel{layer + 1}", [128, lay["total_slots"]],
                                    F32, kind="ExternalInput"))
        t_scl.append(nc.dram_tensor(f"scl{layer + 1}", [128, lay["total_slots"]],
                                    F32, kind="ExternalInput"))
    t_out = nc.dram_tensor("out", [SHARD, DOUT], F32, kind="ExternalOutput")

    max_run_slots = max(
        int(lay["run_len"][g, c]) // 128
        for lay in layouts for g in range(NG) for c in range(4)
    )

    # first/last nonempty chunk per class for layer-2 accumulate
    first_c2 = [min(c for c in range(4) if Ls[1][b, c] > 0) for b in range(BPC)]
    last_c2 = [max(c for c in range(4) if Ls[1][b, c] > 0) for b in range(BPC)]

    with tile.TileContext(nc) as tc:
        with (
            tc.tile_pool(name="const", bufs=1) as constp,
            tc.tile_pool(name="dram", bufs=1, space="DRAM") as dramp,
            tc.tile_pool(name="idxs", bufs=4) as idxp,
            tc.tile_pool(name="msg", bufs=6) as msgp,
            tc.tile_pool(name="ind", bufs=6) as indp,
            tc.tile_pool(name="aggps", bufs=4, space="PSUM") as aggpsp,
            tc.tile_pool(name="xfps", bufs=2, space="PSUM") as xfpsp,
            tc.tile_pool(name="post", bufs=4) as postp,
            tc.tile_pool(name="acc", bufs=1) as accp,
        ):
            nc.gpsimd.load_library(library_config.mlp)

            IOTA = constp.tile([128, 128], BF16)
            nc.sync.dma_start(IOTA[:], t_iota[:, :])
            W1 = constp.tile([DIN, DH], BF16)
            nc.sync.dma_start(W1[:], t_w1[:, :])
            W2 = constp.tile([DH, DOUT], BF16)
            nc.sync.dma_start(W2[:], t_w2[:, :])
            B1R = constp.tile([1, DH], BF16)
            nc.sync.dma_start(B1R[:], t_b1[:, :])
            B2R = constp.tile([1, DOUT], BF16)
            nc.sync.dma_start(B2R[:], t_b2[:, :])
            ONES1 = constp.tile([1, 128], BF16)
            nc.sync.dma_start(ONES1[:], t_ones[:, :])
            DINV2 = constp.tile([128, BPC], F32)
            nc.sync.dma_start(DINV2[:], t_dinv2[:, :])
            SELFSEL = constp.tile([128, 1], F32)
            nc.sync.dma_start(SELFSEL[:], t_selfsel[:, :])
            XSELF = constp.tile([DIN, SHARD], BF16)
            nc.sync.dma_start(XSELF[:], t_xself[:, :])
            SEL = []
            SCL = []
            for layer in (0, 1):
                s = constp.tile([128, layouts[layer]["total_slots"]], F32,
                                name=f"selbuf{layer}")
                nc.sync.dma_start(s[:], t_sel[layer][:, :])
                SEL.append(s)
                s = constp.tile([128, layouts[layer]["total_slots"]], F32,
                                name=f"sclbuf{layer}")
                nc.sync.dma_start(s[:], t_scl[layer][:, :])
                SCL.append(s)

            ACC2 = accp.tile([128, BPC * 128], F32, name="acc2")

            h1_mine = [dramp.tile([QN[q], DH], BF16, name=f"h1mine{q}")
                       for q in range(4)]
            h1_tab = [dramp.tile([QN[q] * NC, DH], BF16, addr_space="Shared",
                                 name=f"h1tab{q}") for q in range(4)]

            gq = [0]

            def do_gather(layer, g, c, src_ap):
                """Load idx + gather the (g, c) run; returns (msg tile, base slot)."""
                lay = layouts[layer]
                rp0 = int(lay["run_base"][g, c])
                nidx = int(lay["run_len"][g, c])
                idxt = idxp.tile([128, (max_run_slots * 128) // 16], I16, tag="idxt")
                nc.sync.dma_start(idxt[:, : nidx // 16],
                                  t_idx[layer][:, rp0 // 16: (rp0 + nidx) // 16])
                msg = msgp.tile([128, max_run_slots, DH], BF16, tag="msg")
                for s0 in range(0, nidx, cfg["GCAP"]):
                    n = min(cfg["GCAP"], nidx - s0)
                    nc.gpsimd.dma_gather(
                        out_ap=msg[:, s0 // 128: (s0 + n) // 128, :],
                        in_ap=src_ap,
                        idxs_ap=idxt[:, s0 // 16: (s0 + n) // 16],
                        num_idxs=n,
                        num_idxs_reg=n,
                        elem_size=DH,
                        queue_num=gq[0] % cfg["NQ"],
                    )
                    gq[0] += 1
                return msg, rp0 // 128

            def agg_slots(layer, b, c, msg, base_slot, psA, start, stop):
                """Indicator matmuls for block b's (b, c) slots into psA.
                start applies to the first matmul, stop to the last."""
                lay = layouts[layer]
                L = Ls[layer]
                s0 = int(lay["gpos"][b, c])
                slots = list(range(s0 // 128, (s0 + L[b, c]) // 128))
                for k, s in enumerate(slots):
                    ind = indp.tile([128, 128], BF16, tag="ind")
                    nc.any.tensor_scalar(
                        ind[:], IOTA[:], SEL[layer][:, s: s + 1],
                        SCL[layer][:, s: s + 1],
                        mybir.AluOpType.is_equal, mybir.AluOpType.mult,
                    )
                    nc.tensor.matmul(
                        psA[:], lhsT=msg[:, s - base_slot, :], rhs=ind[:],
                        start=(start and k == 0),
                        stop=(stop and k == len(slots) - 1),
                    )

            AFT = mybir.ActivationFunctionType

            # ================= layer 1 (block-major) =================
            for g in range(NG):
                msgs = {}
                for c in range(4):
                    if int(layouts[0]["run_len"][g, c]) == 0:
                        continue
                    src_ap = t_xt[c * CH1:(c + 1) * CH1, :]
                    msgs[c] = do_gather(0, g, c, src_ap)
                for b in range(g * BPG, (g + 1) * BPG):
                    chunks = [c for c in range(4) if Ls[0][b, c] > 0]
                    psH = xfpsp.tile([128, DH], F32, tag="xfps")
                    nc.tensor.matmul(psH[:], lhsT=XSELF[:, b * 128:(b + 1) * 128],
                                     rhs=W1[:], start=True, stop=False)
                    if chunks:
                        psA = aggpsp.tile([DH, 128], F32, tag="aggps")
                        for k, c in enumerate(chunks):
                            msg, base = msgs[c]
                            agg_slots(0, b, c, msg, base, psA,
                                      start=(k == 0), stop=(k == len(chunks) - 1))
                        aggs = postp.tile([DH, 128], BF16, tag="aggs")
                        nc.scalar.activation(aggs[:], psA[:], AFT.Copy)
                        nc.tensor.matmul(psH[:], lhsT=aggs[:], rhs=W1[:],
                                         start=False, stop=False)
                    nc.tensor.matmul(psH[:], lhsT=ONES1[:], rhs=B1R[:],
                                     start=False, stop=True)
                    h1s = postp.tile([128, DH], BF16, tag="h1s")
                    nc.scalar.activation(h1s[:], psH[:], AFT.Relu)
                    q = int(_block_quarter(cfg, b))
                    r0 = (b - int(QSTART[q])) * 128
                    nc.sync.dma_start(h1_mine[q][r0:r0 + 128, :], h1s[:])
                    if b == int(QSTART[q + 1]) - 1:
                        if sim_single or fake_ag:
                            nc.sync.dma_start(h1_tab[q][:QN[q], :], h1_mine[q][:, :])
                        else:
                            nc.gpsimd.collective_compute(
                                "AllGather",
                                mybir.AluOpType.bypass,
                                replica_groups=[list(range(NC))],
                                ins=[h1_mine[q].opt()],
                                outs=[h1_tab[q].opt()],
                            )
                del msgs

            # ================= layer 2 (chunk-major) =================
            for c in range(4):
                for g in range(NG):
                    if int(layouts[1]["run_len"][g, c]) == 0:
                        continue
                    msg, base = do_gather(1, g, c, h1_tab[c][:, :])
                    for b in range(g * BPG, (g + 1) * BPG):
                        if Ls[1][b, c] == 0:
                            continue
                        psA = aggpsp.tile([DH, 128], F32, tag="aggps")
                        is_last = last_c2[b] == c
                        agg_slots(1, b, c, msg, base, psA, start=True,
                                  stop=not is_last)
                        if is_last:
                            # self-loop: own h1 rows (local, no AllGather)
                            # aggregated via identity indicator * dinv^2
                            q = int(_block_quarter(cfg, b))
                            r0 = (b - int(QSTART[q])) * 128
                            selfm = postp.tile([128, DH], BF16, tag="selfm")
                            nc.sync.dma_start(selfm[:],
                                              h1_mine[q][r0:r0 + 128, :])
                            sind = indp.tile([128, 128], BF16, tag="ind")
                            nc.any.tensor_scalar(
                                sind[:], IOTA[:], SELFSEL[:, 0:1],
                                DINV2[:, b: b + 1],
                                mybir.AluOpType.is_equal, mybir.AluOpType.mult,
                            )
                            nc.tensor.matmul(psA[:], lhsT=selfm[:], rhs=sind[:],
                                             start=False, stop=True)
                        asl = ACC2[:, b * 128:(b + 1) * 128]
                        if first_c2[b] == c:
                            nc.scalar.activation(asl, psA[:], AFT.Copy)
                        else:
                            nc.vector.tensor_tensor(asl, psA[:], asl,
                                                    mybir.AluOpType.add)

            for b in range(BPC):
                aggs2 = postp.tile([DH, 128], BF16, tag="aggs2")
                nc.scalar.activation(aggs2[:], ACC2[:, b * 128:(b + 1) * 128],
                                     AFT.Copy)
                psO = xfpsp.tile([128, DOUT], F32, tag="xfps2")
                nc.tensor.matmul(psO[:], lhsT=aggs2[:], rhs=W2[:],
                                 start=True, stop=False)
                nc.tensor.matmul(psO[:], lhsT=ONES1[:], rhs=B2R[:],
                                 start=False, stop=True)
                ot = postp.tile([128, DOUT], F32, tag="ot")
                nc.scalar.activation(ot[:], psO[:], AFT.Copy)
                nc.sync.dma_start(t_out[b * 128:(b + 1) * 128, :], ot[:])

    nc.compile()
    return nc


def kernel(x, edge_index, W1, b1, W2, b2):
    cfg = make_cfg(100000, 1600000)
    in_maps, Ls, layouts, node_pos = preprocess(cfg, x, edge_index, W1, b1, W2, b2)
    nc = build_nc(cfg, Ls, layouts, debug=False)
    from concourse import bass_utils
    res = bass_utils.run_bass_kernel_spmd(
        nc, in_maps, core_ids=list(range(cfg["NCORES"]))
    )
    out = np.concatenate([res.results[r]["out"] for r in range(cfg["NCORES"])], axis=0)
    return np.ascontiguousarray(out[node_pos])
